# revision 46
# baseline (speedup 1.0000x reference)
"""Trainium2 Bass kernel for nn_NeighborhoodAttentionModule.

Pure data-parallel over batch: B=16384 rows split as 2048 rows/core across 8
NeuronCores, 16 b-tiles of 128 rows per core. Per b-tile:

  s1T[b,(h,a)]   = ceT8.T @ U2         (PE DoubleRow fp8, 1 matmul)
  z[(h,a),(b,k)] = VU.T @ neT8 + s1T-broadcast   (PE: fp8 DoubleRow + fp16
                   rank-expand matmul accumulated into same PSUM)
  h = tanh(z)                          (ACT, PSUM->SBUF fp16)
  raw[(b,k),(c,h)] = h_chunk.T @ w2    (PE chunk-stationary, 16 tiny matmuls)
  rawn = raw + nwv                     (DVE; nwv = valid ? nw : -30, host-folded)
  em = exp(rawn)                       (ACT fp16; invalid -> exp(-30+raw) == 0)
  S[(b',h),(c,h)] via bm8 matmul; recS = 1/(S+2e-5)  (PE + DVE)
  recSmap = bm8T @ recS                (PE partition-broadcast)
  p = em * recSmap; expblk = p * blockmask           (DVE, fp16)
  aggT[dd,(b,h)] += ner8_chunk.T @ expblk_chunk      (PE, fp8 x fp16 mixed)
  fused = aggT.T @ Wcc + bc            (PE)
  out = LayerNorm(fused + center)      (DVE only: STT-accum stats + int
                                        rsqrt bithack + 2 Newton steps)

Zero ACT table swaps (Tanh+Exp share the exp_and_others table). All DRAM
tensors are laid out host-side as per-tile SBUF images (4KB/512B contiguous
partition lines) for dense DMA descriptors.
"""
import os
import numpy as np

B, K, D, H, A = 16384, 16, 256, 2, 64
TBLOB = 8736          # per-tile input blob bytes per partition
CBLOB = 1946          # fp16 const blob columns
NCORES = 8
BC = B // NCORES      # rows per core (2048)
NBT = BC // 128       # b-tiles per core (16)
NCH = 16              # chunks of 128 (b,k)-rows per b-tile
HA = H * A            # 128
EPS = 1e-5
NWV_NEG = -30.0       # folded invalid-neighbor bias
S_EPS = 2e-5          # S regularizer (valid rows always have S >= 1.5e-3)
RSQRT_MAGIC = 0x5F3759DF

LAST_EXEC_NS = None

_prog_cache = {}


def _maybe_install_profile_hook():
    """Optional NTFF profiling hook (for local testing only; fails soft)."""
    import sys, types, contextlib, ctypes

    if "antenv.axon_hooks" in sys.modules:
        return
    try:
        mod = types.ModuleType("antenv.axon_hooks")
        _state = {"hook": None}
        mod.set_axon_ntff_profile_hook = lambda h: _state.__setitem__("hook", h)
        mod.get_axon_ntff_profile_hook = lambda: _state["hook"]
        sys.modules["antenv.axon_hooks"] = mod
        import antenv

        antenv.axon_hooks = mod
        so_path = "/opt/axon/libaxon_pjrt.so"
        lib = ctypes.CDLL(so_path)
        if not hasattr(lib, "axon_start_nrt_profile"):
            return
        lib.axon_start_nrt_profile.argtypes = [
            ctypes.POINTER(ctypes.c_int64),
            ctypes.c_size_t,
        ]
        lib.axon_start_nrt_profile.restype = ctypes.c_int64
        lib.axon_stop_nrt_profile.argtypes = [ctypes.c_char_p]
        lib.axon_stop_nrt_profile.restype = ctypes.c_int64

        @contextlib.contextmanager
        def _hook(output_dir, device_ids):
            import jax

            jax.devices()
            if device_ids:
                ids = (ctypes.c_int64 * len(device_ids))(*device_ids)
                rc = lib.axon_start_nrt_profile(ids, len(device_ids))
            else:
                rc = lib.axon_start_nrt_profile(None, 0)
            if rc != 0:
                raise RuntimeError(f"axon_start_nrt_profile rc={rc}")
            try:
                yield
            finally:
                n = lib.axon_stop_nrt_profile(str(output_dir).encode())
                print(f"profile: {n} ntff file(s) -> {output_dir}")

        mod.set_axon_ntff_profile_hook(_hook)
    except Exception as e:  # noqa: BLE001
        print("profile hook unavailable:", e)


def _build_program(apply_gamma_beta: bool, apply_b1: bool):
    from concourse import bacc, tile, mybir

    F8 = mybir.dt.float8e4
    F16 = mybir.dt.float16
    F32 = mybir.dt.float32
    I32 = mybir.dt.int32
    AFT = mybir.ActivationFunctionType
    ALU = mybir.AluOpType
    PM = mybir.MatmulPerfMode

    nc = bacc.Bacc(None, target_bir_lowering=False)

    # ---- DRAM parameters (per-core shard; all per-tile SBUF images) ----
    dp = nc.declare_dram_parameter
    # per-tile blob: neT8 4096B | ner8 4096B | ce16 512B | nwv16 32B
    blob_d = dp("blob8", [NBT, 128, TBLOB], F8, isOutput=False)
    ceT_d = dp("ceT8", [128, 2, BC], F8, isOutput=False)
    vu2_d = dp("vu2_8", [128, 2, 2 * HA], F8, isOutput=False)  # vu | u2
    # fp16 const blob: w2p 2 | bm8 8 | bm16 16 | wcc 1024 | bm8T 128 |
    # idk 512 | ones 128 | b1 128
    cb_d = dp("cblob16", [128, CBLOB], F16, isOutput=False)
    gam_d = dp("gamma_r", [1, D], F32, isOutput=False)
    bet_d = dp("beta_r", [1, D], F32, isOutput=False)
    out_d = dp("out", [BC, D], F32, isOutput=True)
    debug = bool(os.environ.get("NE_DEBUG_DUMP"))
    if debug:
        dbg_h_d = dp("dbg_h", [128, 2048], F16, isOutput=True)
        dbg_rawn_d = dp("dbg_rawn", [128, NCH, H], F32, isOutput=True)
        dbg_em_d = dp("dbg_em", [128, NCH, H], F16, isOutput=True)
        dbg_p_d = dp("dbg_p", [128, NCH, H], F16, isOutput=True)
        dbg_aggT_d = dp("dbg_aggT", [128, 2, 2 * 128], F16, isOutput=True)
        dbg_x_d = dp("dbg_x", [128, D], F32, isOutput=True)

    with tile.TileContext(nc) as tc:
        with (
            tc.tile_pool(name="const", bufs=1) as cpool,
            tc.tile_pool(name="loads", bufs=6) as lpool,
            tc.tile_pool(name="work", bufs=2) as wpool,
            tc.tile_pool(name="xs", bufs=12) as xpool,
            tc.tile_pool(name="gchain", bufs=4) as gpool,
            tc.tile_pool(name="zps", bufs=2, space="PSUM") as zps_p,
            tc.tile_pool(name="srm_ps", bufs=2, space="PSUM") as srm_p,
            tc.tile_pool(name="aggps", bufs=1, space="PSUM") as aggps_p,
            tc.tile_pool(name="fups", bufs=1, space="PSUM") as fups_p,
        ):
            def cload(name, dram_ap, shape, dt):
                t = cpool.tile(shape, dt, tag=name, name=name)
                nc.sync.dma_start(t[:], dram_ap)
                return t

            # vu2 first: it is tiny and, with blob(0), is all z-vu(0) needs.
            # ceT8 (0.5MB) follows (z-u2 needs it ~1us later); cb after.
            vu2 = cload("vu2", vu2_d[:], [128, 2, 2 * HA], F8)
            vu8 = vu2[:, :, 0:HA]
            u28 = vu2[:, :, HA:2 * HA]
            ceT8 = cpool.tile([128, 2, BC], F8, tag="ceT8", name="ceT8")
            cb = cpool.tile([128, CBLOB], F16, tag="cb", name="cb")
            w2p = cb[:, 0:2]
            bm8 = cb[:, 2:10]
            bm16 = cb[:, 10:26].rearrange("p (b h) -> p b h", h=H)
            wcc = [[cb[:, 26 + (h * 2 + dh) * D:26 + (h * 2 + dh + 1) * D]
                    for dh in range(2)] for h in range(2)]
            bm8T = cb[0:8, 1050:1178]
            if apply_b1:
                b1c = cpool.tile([128, 1], F32, tag="b1c")
                nc.vector.tensor_copy(b1c[:], cb[:, 1818:1819])
            gam_t = (
                cload("gam", gam_d[:].to_broadcast((128, D)), [128, D], F32)
                if apply_gamma_beta else None
            )
            bet_t = (
                cload("bet", bet_d[:].to_broadcast((128, D)), [128, D], F32)
                if apply_gamma_beta else None
            )

            def issue_loads(t):
                blob = lpool.tile([128, TBLOB], F8, tag="blob")
                nc.sync.dma_start(blob[:], blob_d[t])
                neT = blob[:, 0:4096].rearrange("p (i c) -> p i c", i=2)
                ner = blob[:, 4096:8192].rearrange("p (c d) -> p c d", c=NCH)
                cen = blob[:, 8192:8704].bitcast(F16)
                nwv = blob[:, 8704:8736].bitcast(F16)
                return neT, ner, cen, nwv

            def issue_fused(t, aggT, cen, ps=None):
                # fused = combined @ Wc (bc folded into cen host-side)
                fu_ps = ps if ps is not None else fups_p.tile(
                    [128, D], F32, tag="fu")
                mms = [(h, dh) for h in range(2) for dh in range(2)]
                for i, (h, dh) in enumerate(mms):
                    lhs = aggT[:, dh].rearrange("p (b h) -> p h b", h=2)[:, h, :]
                    nc.tensor.matmul(
                        fu_ps[:], lhs, wcc[h][dh],
                        start=(i == 0), stop=(i == 3),
                    )
                return (t, fu_ps, cen)

            # LN stats staged per-tile into columns; the tiny rsqrt chain
            # runs BATCHED (4 tiles per group) on the otherwise-idle GPSIMD
            # engine so it never clogs the DVE queue (the rawn->exp->S
            # critical chain was stalling ~1us/tile behind it).
            GRP = 4
            xts = {}
            gstats = {}

            def issue_ln_pre(t, fu_ps, cen):
                # residual add + sum / sumsq accumulation (DVE)
                g = t // GRP
                if g not in gstats:
                    gstats[g] = (
                        gpool.tile([128, GRP], F32, tag="msum_g",
                                   name="msum_g"),
                        gpool.tile([128, GRP], F32, tag="sumsq_g",
                                   name="sumsq_g"),
                        gpool.tile([128, GRP], F32, tag="negm_g",
                                   name="negm_g"),
                        gpool.tile([128, GRP], I32, tag="inv_g",
                                   name="inv_g"),
                    )
                msum_g, sumsq_g, _, _ = gstats[g]
                i = t % GRP
                x_t = xpool.tile([128, D], F32, tag="x")
                xts[t] = x_t
                nc.vector.scalar_tensor_tensor(
                    x_t[:], fu_ps[:], 1.0, cen[:],
                    op0=ALU.mult, op1=ALU.add, accum_out=msum_g[:, i:i + 1],
                )
                if debug and t == 0:
                    nc.sync.dma_start(dbg_x_d[:], x_t[:])
                sq_t = wpool.tile([128, D], F32, tag="sq")
                nc.vector.scalar_tensor_tensor(
                    sq_t[:], x_t[:], 1.0, x_t[:],
                    op0=ALU.mult, op1=ALU.mult,
                    accum_out=sumsq_g[:, i:i + 1],
                )

            def issue_ln_chain(g, dve=False):
                # invstd = rsqrt(sumsq/D - mean^2 + eps) for 4 tiles at once.
                # Seed comes from sumsq/D + eps alone (the mean^2 term is
                # ~1% of q for LN inputs; Newton recovers it) so the DVE ops
                # here never depend on GPSIMD results. The exact q and the
                # Newton steps run on the otherwise-idle GPSIMD.
                msum_g, sumsq_g, negm_g, inv_g = gstats[g]
                negm = negm_g[:]
                yi = inv_g[:]
                q_t = gpool.tile([128, GRP], F32, tag="qg")
                qh = gpool.tile([128, GRP], F32, tag="qhg")
                m2 = gpool.tile([128, GRP], F32, tag="m2g")
                nr1 = gpool.tile([128, GRP], F32, tag="nr1g")
                nr2 = gpool.tile([128, GRP], F32, tag="nr2g")
                nc.vector.tensor_scalar(
                    q_t[:], sumsq_g[:], 1.0 / D, EPS,
                    op0=ALU.mult, op1=ALU.add,
                )
                nc.vector.tensor_scalar(
                    yi, q_t[:].bitcast(I32), 1, None,
                    op0=ALU.logical_shift_right,
                )
                nc.vector.tensor_scalar(
                    yi, yi, RSQRT_MAGIC, -1, op0=ALU.subtract, op1=ALU.mult,
                )
                E = nc.vector if dve else nc.gpsimd
                E.tensor_scalar_mul(negm, msum_g[:], -1.0 / D)
                E.tensor_mul(m2[:], negm, negm)
                E.tensor_sub(q_t[:], q_t[:], m2[:])
                E.tensor_scalar_mul(qh[:], q_t[:], -0.5)
                y = yi.bitcast(F32)
                for _ in range(2):
                    E.tensor_mul(nr1[:], y, y)
                    E.tensor_mul(nr2[:], qh[:], nr1[:])
                    E.tensor_scalar(nr1[:], nr2[:], 1.5, None, op0=ALU.add)
                    E.tensor_mul(yi.bitcast(F32), y, nr1[:])

            def issue_ln_post(t):
                g = t // GRP
                i = t % GRP
                _, _, negm_g, inv_g = gstats[g]
                x_t = xts.pop(t)
                xn = wpool.tile([128, D], F32, tag="xn")
                nc.vector.tensor_scalar(
                    xn[:], x_t[:], negm_g[:, i:i + 1],
                    inv_g[:, i:i + 1].bitcast(F32),
                    op0=ALU.add, op1=ALU.mult,
                )
                if apply_gamma_beta:
                    nc.vector.tensor_mul(xn[:], xn[:], gam_t[:])
                    nc.vector.tensor_add(xn[:], xn[:], bet_t[:])
                nc.sync.dma_start(out_d[t * 128:(t + 1) * 128, :], xn[:])

            def issue_z_vu(t, neT, hf):
                # z partial: VU.T @ neT8, fp8 DoubleRow, 512-col MMs (one
                # PSUM bank each). All vu MMs for both halves are grouped so
                # the DoubleRow weights load only twice per tile.
                z_ps = zps_p.tile([128, 1024], F32, tag="z")
                for m in range(2):
                    c0 = hf * 1024 + m * 512
                    nc.tensor.matmul(
                        z_ps[:, m * 512:(m + 1) * 512], vu8,
                        neT[:, :, c0:c0 + 512],
                        start=True, stop=False, perf_mode=PM.DoubleRow,
                        skip_group_check=True,
                    )
                return z_ps

            def issue_z_u2(t, z_ps, hf):
                # z accumulate: U2.T @ ceT8-kexp (stride-0 k-broadcast AP)
                for m in range(2):
                    b0 = t * 128 + hf * 64 + m * 32
                    ce_kexp = ceT8[:, :, b0:b0 + 32][:, :, :, None] \
                        .to_broadcast((128, 2, 32, 16))
                    nc.tensor.matmul(
                        z_ps[:, m * 512:(m + 1) * 512], u28, ce_kexp,
                        start=False, stop=True, perf_mode=PM.DoubleRow,
                        skip_group_check=True,
                    )

            def issue_tanh(h_sb, z_ps, hf):
                if apply_b1:
                    nc.scalar.activation(
                        h_sb[:, hf * 1024:(hf + 1) * 1024], z_ps[:],
                        AFT.Tanh, bias=b1c[:],
                    )
                else:
                    nc.scalar.activation(
                        h_sb[:, hf * 1024:(hf + 1) * 1024], z_ps[:], AFT.Tanh,
                    )

            # ---- PE warmup spin ----
            # The HAM clock gate keeps the PE at 1.2 GHz until it sees
            # ~3.4us of sustained matmul activity. Spin dummy 256-col MMs
            # on scratch data during the initial blob DMA so real work
            # starts at 2.4 GHz. Output goes to the fused PSUM bank and is
            # overwritten (start=True) by the first real fused matmul.
            warm_sb = cpool.tile([128, 256], F8, tag="warm", name="warm")
            nc.gpsimd.memset(warm_sb[:], 0)
            warm_ps = fups_p.tile([128, 256], F32, tag="fu")
            for _ in range(24):
                nc.tensor.matmul(
                    warm_ps[:], warm_sb[:, 0:128], warm_sb[:],
                    start=True, stop=True, skip_group_check=True,
                )

            def issue_raw(t, h_sb):
                # raw scores (chunk-stationary), + nwv add, + exp.
                # Issued at the tail of iteration t-1 so em(t) is ready
                # before S(t) at the start of iteration t.
                srm = srm_p.tile([128, NCH, H], F32, tag="srm")
                for c in range(NCH):
                    nc.tensor.matmul(
                        srm[:, c, :],
                        h_sb[:, c * 128:(c + 1) * 128], w2p,
                        start=True, stop=True,
                    )
                nwv = tiles[t][3]
                rawn = wpool.tile([128, NCH, H], F32, tag="rawn")
                nc.vector.tensor_add(
                    rawn[:], srm[:],
                    nwv[:, :, None].to_broadcast((128, NCH, H)),
                )
                if debug and t == 0:
                    nc.sync.dma_start(dbg_rawn_d[:], rawn[:])
                em = wpool.tile([128, NCH, H], F16, tag="em")
                nc.scalar.activation(
                    em[:].rearrange("p c h -> p (c h)"),
                    rawn[:].rearrange("p c h -> p (c h)"), AFT.Exp,
                )
                return srm, em

            # ---- software-pipelined main loop ----
            # iteration t: S/softmax/agg/copy for tile t, fused+LN-pre for
            # t-2, z+tanh for t+1, raw+exp for t+1 at the tail (after agg,
            # whose 32 MMs hide the tanh-h1 latency). em(t) is produced at
            # the previous iteration's tail so S(t) never waits; the aggT
            # copy goes last in the ACT queue (it isn't needed for 2 more
            # iterations) so exp(t+1) retires promptly.
            tiles = {0: issue_loads(0)}
            nc.sync.dma_start(ceT8[:], ceT_d[:])
            nc.sync.dma_start(cb[:], cb_d[:])
            tiles[1] = issue_loads(1)
            hs = {0: wpool.tile([128, 2048], F16, tag="h", name="h")}
            zpa = issue_z_vu(0, tiles[0][0], 0)
            zpb = issue_z_vu(0, tiles[0][0], 1)
            issue_z_u2(0, zpa, 0)
            issue_tanh(hs[0], zpa, 0)
            issue_z_u2(0, zpb, 1)
            issue_tanh(hs[0], zpb, 1)
            if debug:
                nc.sync.dma_start(dbg_h_d[:], hs[0][:])
            srm_cur, em_cur = issue_raw(0, hs[0])
            pending = []
            ready_posts = []
            copy_pending = None
            for t in range(NBT):
                neT, ner, cen, nwv = tiles[t]
                if t + 2 < NBT:
                    tiles[t + 2] = issue_loads(t + 2)
                srm, em = srm_cur, em_cur

                ln_args = None
                if len(pending) == 2:
                    ln_args = issue_fused(*pending.pop(0))
                # aggT copy of tile t-1, deferred to AFTER fused(t-2)'s
                # issue: the scheduler then orders it late in the ACT queue,
                # so fused's (coalesced) ACT-sem wait never covers it. A
                # full iteration remains before fused(t-1) needs it.
                if copy_pending is not None:
                    nc.scalar.copy(copy_pending[0][:], copy_pending[1][:])
                    copy_pending = None

                s_ps = srm[0:8].rearrange("p c h -> p (c h)")
                nc.tensor.matmul(
                    s_ps, bm8, em[:].rearrange("p c h -> p (c h)"),
                    start=True, stop=True,
                )
                s_eps = wpool.tile([8, NCH * H], F32, tag="seps")
                nc.vector.tensor_scalar_add(s_eps[:], s_ps, S_EPS)
                recS = wpool.tile([8, NCH * H], F16, tag="recS")
                with nc.allow_low_precision(reason="recS feeds fp16 matmul"):
                    nc.vector.reciprocal(recS[:], s_eps[:])

                # z-vu for both halves of t+1 fills the recS latency on PE
                # (single DoubleRow weight load for all 4 MMs)
                zp1 = zp2 = None
                if t + 1 < NBT:
                    hs[t + 1] = wpool.tile([128, 2048], F16, tag="h",
                                           name="h")
                    zp1 = issue_z_vu(t + 1, tiles[t + 1][0], 0)
                    zp2 = issue_z_vu(t + 1, tiles[t + 1][0], 1)

                rmap_ps = srm[:].rearrange("p c h -> p (c h)")
                nc.tensor.matmul(rmap_ps, bm8T, recS[:], start=True, stop=True)

                # z-u2 for both halves fills the p/expblk latency before agg
                # (single u2 weight load); tanh after each half completes
                if t + 1 < NBT:
                    issue_z_u2(t + 1, zp1, 0)
                    issue_tanh(hs[t + 1], zp1, 0)
                    issue_z_u2(t + 1, zp2, 1)
                    issue_tanh(hs[t + 1], zp2, 1)

                p_sb = wpool.tile([128, NCH, H], F16, tag="p")
                nc.vector.tensor_mul(p_sb[:], em[:], srm[:])
                if debug and t == 0:
                    nc.sync.dma_start(dbg_em_d[:], em[:])
                    nc.sync.dma_start(dbg_p_d[:], p_sb[:])
                expblk = wpool.tile([128, NCH, 8, H], F16, tag="expblk")
                for half in range(2):
                    hc = slice(half * (NCH // 2), (half + 1) * (NCH // 2))
                    nc.vector.tensor_mul(
                        expblk[:, hc],
                        p_sb[:, hc, None, :].to_broadcast((128, NCH // 2, 8, H)),
                        bm16[:, None, :, :].to_broadcast((128, NCH // 2, 8, H)),
                    )
                chain_g = None
                if ln_args is not None:
                    issue_ln_pre(*ln_args)
                    tp = ln_args[0]
                    if tp % GRP == GRP - 1:
                        chain_g = tp // GRP
                # last iteration: pull fused(NBT-2) forward (its ln_pre for
                # NBT-4 is already issued above, so the fu_ps WAR is safe)
                ln_extra = None
                if t == NBT - 1 and pending:
                    ln_extra = issue_fused(*pending.pop(0))
                # ---- aggT[dd, dh, (b,h)] += ner8_c.T @ expblk_c ----
                agg_ps = aggps_p.tile([128, 2, 2 * 128], F32, tag="aggT")
                for c in range(NCH):
                    for dh in range(2):
                        nc.tensor.matmul(
                            agg_ps[:, dh, 16 * c:16 * c + 16],
                            ner[:, c, dh * 128:(dh + 1) * 128],
                            expblk[:, c],
                            start=True, stop=True,
                        )
                # raw/exp for t+1 at the tail: agg's MMs just filled the
                # tanh(t+1,h1) latency, so raw doesn't stall
                if t + 1 < NBT:
                    srm_cur, em_cur = issue_raw(t + 1, hs[t + 1])
                # rsqrt chain + xn posts AFTER rawn(t+1) in the DVE queue:
                # on group-boundary iterations their seed ops were delaying
                # rawn -> exp -> S(t+1) by ~0.8us, which tripped the PE
                # clock gate for the rest of the kernel
                if chain_g is not None:
                    issue_ln_chain(chain_g)
                    # delay posts 2 iterations so the GPSIMD chain is
                    # long done before any DVE op waits on it
                    for j, tpost in enumerate(
                            range(GRP * chain_g, GRP * chain_g + GRP)):
                        ready_posts.append((t + 2 + j, tpost))
                while ready_posts and ready_posts[0][0] <= t:
                    issue_ln_post(ready_posts.pop(0)[1])
                aggT = wpool.tile([128, 2, 2 * 128], F16, tag="aggTsb", bufs=3)
                copy_pending = (aggT, agg_ps)
                if debug and t == 0:
                    nc.sync.dma_start(dbg_aggT_d[:], aggT[:])
                tiles.pop(t - 2, None)
                hs.pop(t - 1, None)
                pending.append((t, aggT, cen))

            # drain: ln_pre for NBT-2 (its fused was pulled into the last
            # iteration), then fused(NBT-1) into a now-free z PSUM bank (so
            # it doesn't wait on the fu_ps WAR), the last rsqrt chain on DVE
            # (GPSIMD dispatch is slow and nothing else needs DVE now), and
            # the remaining posts oldest-first.
            if copy_pending is not None:
                nc.scalar.copy(copy_pending[0][:], copy_pending[1][:])
                copy_pending = None
            if ln_extra is not None:
                issue_ln_pre(*ln_extra)
            for args in pending:
                fu15 = zps_p.tile([128, 1024], F32, tag="z", name="fu15")
                issue_ln_pre(*issue_fused(*args, ps=fu15[:, 0:D]))
                tp = args[0]
                if tp % GRP == GRP - 1:
                    issue_ln_chain(tp // GRP, dve=True)
                    for tpost in range(tp - GRP + 1, tp + 1):
                        ready_posts.append((0, tpost))
            for _, tp in ready_posts:
                issue_ln_post(tp)

    nc.finalize()
    return nc


def _f8(x):
    import ml_dtypes
    return np.clip(x, -240.0, 240.0).astype(ml_dtypes.float8_e4m3)


def _patch_ldw_opt():
    import concourse.bass_utils as _bu
    if getattr(_bu, "_ldwopt_patched", False):
        return
    _bu._ldwopt_patched = True


def kernel(center_emb, neighbor_embs, neighbor_weights, neighbor_valid,
           W1, b1, w2, Wc, bc, alpha, gamma, beta):
    _patch_ldw_opt()
    from concourse.bass_utils import run_bass_kernel_spmd

    global LAST_EXEC_NS

    f32 = np.float32
    f16 = np.float16
    ce = np.asarray(center_emb, f32)
    ne = np.asarray(neighbor_embs, f32)
    nw = np.asarray(neighbor_weights, f32)
    va = np.asarray(neighbor_valid)
    W1 = np.asarray(W1, f32)
    b1 = np.asarray(b1, f32)
    w2 = np.asarray(w2, f32)
    Wc = np.asarray(Wc, f32)
    bc = np.asarray(bc, f32)
    alpha = np.asarray(alpha, f32)
    gamma = np.asarray(gamma, f32)
    beta = np.asarray(beta, f32)

    apply_gamma_beta = not (np.all(gamma == 1.0) and np.all(beta == 0.0))
    apply_b1 = bool(np.any(b1 != 0.0))

    key = (apply_gamma_beta, apply_b1, bool(os.environ.get("NE_DEBUG_DUMP")))
    if key not in _prog_cache:
        _prog_cache[key] = _build_program(key[0], key[1])
    nc = _prog_cache[key]

    # ---- host-side const prep (weight folding + dtype casts + layouts) ----
    import ml_dtypes
    F8NP = ml_dtypes.float8_e4m3
    sig = 1.0 / (1.0 + np.exp(-float(alpha[0])))
    VU = np.concatenate([W1[h, D:2 * D] - W1[h, 2 * D:3 * D] for h in range(H)], axis=1)
    U2 = np.concatenate([W1[h, :D] + W1[h, 2 * D:3 * D] for h in range(H)], axis=1)
    # d = p + 128*i  ->  [p, i, cols];  vu | u2 side by side
    vu2 = np.concatenate([
        _f8(VU).reshape(2, 128, HA).transpose(1, 0, 2),
        _f8(U2).reshape(2, 128, HA).transpose(1, 0, 2),
    ], axis=2)
    vu2 = np.ascontiguousarray(vu2)

    cb = np.zeros((128, CBLOB), f16)
    for h in range(H):
        cb[h * A:(h + 1) * A, h] = w2[h].astype(f16)          # w2p [*,0:2]
    pidx = np.arange(128)
    cb[:, 2:10] = (pidx[:, None] // 16 == np.arange(8)[None, :])   # bm8
    for p in range(128):
        cb[p, 10 + (p // 16) * H:10 + (p // 16) * H + H] = 1.0     # bm16
    wcc_f = (Wc * sig).astype(f16).reshape(H, 2, 128, D)
    for h in range(H):
        for dh in range(2):
            c0 = 26 + (h * 2 + dh) * D
            cb[:, c0:c0 + D] = wcc_f[h, dh]
    cb[0:8, 1050:1178] = (pidx[None, :] // 16 == np.arange(8)[:, None])  # bm8T
    for pl in range(32):
        cb[pl, 1178 + pl * 16:1178 + (pl + 1) * 16] = 1.0       # idk
    cb[0, 1690:1818] = 1.0                                       # ones_row
    cb[:, 1818] = b1.reshape(HA).astype(f16)                     # b1 column

    gamma_r = gamma.reshape(1, D).astype(f32)
    beta_r = beta.reshape(1, D).astype(f32)

    nwv = np.where(va, nw, NWV_NEG).astype(f16)        # [B, K]
    ce_bc = (ce + (bc * sig)[None, :]).astype(f16)     # bc folded into center

    in_maps = []
    for cidx in range(NCORES):
        rs = slice(cidx * BC, (cidx + 1) * BC)
        ne_c = _f8(ne[rs].reshape(BC * K, D))          # [BC*K, D] fp8
        blob = np.zeros((NBT, 128, TBLOB), np.uint8)
        # neT8 [t, p, i, col]: ne[row(t,col), p+128i]
        neT8 = np.ascontiguousarray(
            ne_c.reshape(NBT, 2048, 2, 128).transpose(0, 3, 2, 1)
        )
        blob[:, :, 0:4096] = neT8.reshape(NBT, 128, 4096).view(np.uint8)
        # ner8 [t, p, c, d]: ne[t*2048 + c*128 + p, d]
        ner8 = np.ascontiguousarray(
            ne_c.reshape(NBT, NCH, 128, D).transpose(0, 2, 1, 3)
        )
        blob[:, :, 4096:8192] = ner8.reshape(NBT, 128, 4096).view(np.uint8)
        blob[:, :, 8192:8704] = np.ascontiguousarray(
            ce_bc[rs].reshape(NBT, 128, D)).view(np.uint8).reshape(NBT, 128, 512)
        blob[:, :, 8704:8736] = np.ascontiguousarray(
            nwv[rs].reshape(NBT, NCH, 128).transpose(0, 2, 1)
        ).view(np.uint8).reshape(NBT, 128, 32)
        ceT8 = np.ascontiguousarray(
            _f8(ce[rs]).reshape(BC, 2, 128).transpose(2, 1, 0)
        )
        in_maps.append({
            "blob8": blob.view(F8NP),
            "ceT8": ceT8,
            "vu2_8": vu2,
            "cblob16": cb,
            "gamma_r": gamma_r,
            "beta_r": beta_r,
        })

    trace = bool(os.environ.get("NE_KERNEL_TRACE"))
    if trace:
        _maybe_install_profile_hook()
    res = run_bass_kernel_spmd(nc, in_maps, list(range(NCORES)), trace=trace)
    LAST_EXEC_NS = res.exec_time_ns
    if trace:
        print("kernel exec_time_ns:", res.exec_time_ns, "mean:", res.mean_exec_time_ns)

    out = np.empty((B, D), f32)
    for cidx in range(NCORES):
        out[cidx * BC:(cidx + 1) * BC] = res.results[cidx]["out"]
    return out



# revision 47
# speedup vs baseline: 1.0047x; 1.0047x over previous
"""Trainium2 Bass kernel for nn_NeighborhoodAttentionModule.

Pure data-parallel over batch: B=16384 rows split as 2048 rows/core across 8
NeuronCores, 16 b-tiles of 128 rows per core. Per b-tile:

  s1T[b,(h,a)]   = ceT8.T @ U2         (PE DoubleRow fp8, 1 matmul)
  z[(h,a),(b,k)] = VU.T @ neT8 + s1T-broadcast   (PE: fp8 DoubleRow + fp16
                   rank-expand matmul accumulated into same PSUM)
  h = tanh(z)                          (ACT, PSUM->SBUF fp16)
  raw[(b,k),(c,h)] = h_chunk.T @ w2    (PE chunk-stationary, 16 tiny matmuls)
  rawn = raw + nwv                     (DVE; nwv = valid ? nw : -30, host-folded)
  em = exp(rawn)                       (ACT fp16; invalid -> exp(-30+raw) == 0)
  S[(b',h),(c,h)] via bm8 matmul; recS = 1/(S+2e-5)  (PE + DVE)
  recSmap = bm8T @ recS                (PE partition-broadcast)
  p = em * recSmap; expblk = p * blockmask           (DVE, fp16)
  aggT[dd,(b,h)] += ner8_chunk.T @ expblk_chunk      (PE, fp8 x fp16 mixed)
  fused = aggT.T @ Wcc + bc            (PE)
  out = LayerNorm(fused + center)      (DVE only: STT-accum stats + int
                                        rsqrt bithack + 2 Newton steps)

Zero ACT table swaps (Tanh+Exp share the exp_and_others table). All DRAM
tensors are laid out host-side as per-tile SBUF images (4KB/512B contiguous
partition lines) for dense DMA descriptors.
"""
import os
import numpy as np

B, K, D, H, A = 16384, 16, 256, 2, 64
TBLOB = 8736          # per-tile input blob bytes per partition
CBLOB = 1946          # fp16 const blob columns
NCORES = 8
BC = B // NCORES      # rows per core (2048)
NBT = BC // 128       # b-tiles per core (16)
NCH = 16              # chunks of 128 (b,k)-rows per b-tile
HA = H * A            # 128
EPS = 1e-5
NWV_NEG = -30.0       # folded invalid-neighbor bias
S_EPS = 2e-5          # S regularizer (valid rows always have S >= 1.5e-3)
RSQRT_MAGIC = 0x5F3759DF

LAST_EXEC_NS = None

_prog_cache = {}


def _maybe_install_profile_hook():
    """Optional NTFF profiling hook (for local testing only; fails soft)."""
    import sys, types, contextlib, ctypes

    if "antenv.axon_hooks" in sys.modules:
        return
    try:
        mod = types.ModuleType("antenv.axon_hooks")
        _state = {"hook": None}
        mod.set_axon_ntff_profile_hook = lambda h: _state.__setitem__("hook", h)
        mod.get_axon_ntff_profile_hook = lambda: _state["hook"]
        sys.modules["antenv.axon_hooks"] = mod
        import antenv

        antenv.axon_hooks = mod
        so_path = "/opt/axon/libaxon_pjrt.so"
        lib = ctypes.CDLL(so_path)
        if not hasattr(lib, "axon_start_nrt_profile"):
            return
        lib.axon_start_nrt_profile.argtypes = [
            ctypes.POINTER(ctypes.c_int64),
            ctypes.c_size_t,
        ]
        lib.axon_start_nrt_profile.restype = ctypes.c_int64
        lib.axon_stop_nrt_profile.argtypes = [ctypes.c_char_p]
        lib.axon_stop_nrt_profile.restype = ctypes.c_int64

        @contextlib.contextmanager
        def _hook(output_dir, device_ids):
            import jax

            jax.devices()
            if device_ids:
                ids = (ctypes.c_int64 * len(device_ids))(*device_ids)
                rc = lib.axon_start_nrt_profile(ids, len(device_ids))
            else:
                rc = lib.axon_start_nrt_profile(None, 0)
            if rc != 0:
                raise RuntimeError(f"axon_start_nrt_profile rc={rc}")
            try:
                yield
            finally:
                n = lib.axon_stop_nrt_profile(str(output_dir).encode())
                print(f"profile: {n} ntff file(s) -> {output_dir}")

        mod.set_axon_ntff_profile_hook(_hook)
    except Exception as e:  # noqa: BLE001
        print("profile hook unavailable:", e)


def _build_program(apply_gamma_beta: bool, apply_b1: bool):
    from concourse import bacc, tile, mybir

    F8 = mybir.dt.float8e4
    F16 = mybir.dt.float16
    F32 = mybir.dt.float32
    I32 = mybir.dt.int32
    AFT = mybir.ActivationFunctionType
    ALU = mybir.AluOpType
    PM = mybir.MatmulPerfMode

    nc = bacc.Bacc(None, target_bir_lowering=False)

    # ---- DRAM parameters (per-core shard; all per-tile SBUF images) ----
    dp = nc.declare_dram_parameter
    # per-tile blob: neT8 4096B | ner8 4096B | ce16 512B | nwv16 32B
    blob_d = dp("blob8", [NBT, 128, TBLOB], F8, isOutput=False)
    ceT_d = dp("ceT8", [128, 2, BC], F8, isOutput=False)
    vu2_d = dp("vu2_8", [128, 2, 2 * HA], F8, isOutput=False)  # vu | u2
    # fp16 const blob: w2p 2 | bm8 8 | bm16 16 | wcc 1024 | bm8T 128 |
    # idk 512 | ones 128 | b1 128
    cb_d = dp("cblob16", [128, CBLOB], F16, isOutput=False)
    gam_d = dp("gamma_r", [1, D], F32, isOutput=False)
    bet_d = dp("beta_r", [1, D], F32, isOutput=False)
    out_d = dp("out", [BC, D], F32, isOutput=True)
    debug = bool(os.environ.get("NE_DEBUG_DUMP"))
    if debug:
        dbg_h_d = dp("dbg_h", [128, 2048], F16, isOutput=True)
        dbg_rawn_d = dp("dbg_rawn", [128, NCH, H], F32, isOutput=True)
        dbg_em_d = dp("dbg_em", [128, NCH, H], F16, isOutput=True)
        dbg_p_d = dp("dbg_p", [128, NCH, H], F16, isOutput=True)
        dbg_aggT_d = dp("dbg_aggT", [128, 2, 2 * 128], F16, isOutput=True)
        dbg_x_d = dp("dbg_x", [128, D], F32, isOutput=True)

    with tile.TileContext(nc) as tc:
        with (
            tc.tile_pool(name="const", bufs=1) as cpool,
            tc.tile_pool(name="loads", bufs=6) as lpool,
            tc.tile_pool(name="work", bufs=2) as wpool,
            tc.tile_pool(name="xs", bufs=12) as xpool,
            tc.tile_pool(name="gchain", bufs=4) as gpool,
            tc.tile_pool(name="zps", bufs=2, space="PSUM") as zps_p,
            tc.tile_pool(name="srm_ps", bufs=2, space="PSUM") as srm_p,
            tc.tile_pool(name="aggps", bufs=1, space="PSUM") as aggps_p,
            tc.tile_pool(name="fups", bufs=1, space="PSUM") as fups_p,
        ):
            def cload(name, dram_ap, shape, dt):
                t = cpool.tile(shape, dt, tag=name, name=name)
                nc.sync.dma_start(t[:], dram_ap)
                return t

            # vu2 first: it is tiny and, with blob(0), is all z-vu(0) needs.
            # ceT8 (0.5MB) follows (z-u2 needs it ~1us later); cb after.
            vu2 = cload("vu2", vu2_d[:], [128, 2, 2 * HA], F8)
            vu8 = vu2[:, :, 0:HA]
            u28 = vu2[:, :, HA:2 * HA]
            ceT8 = cpool.tile([128, 2, BC], F8, tag="ceT8", name="ceT8")
            cb = cpool.tile([128, CBLOB], F16, tag="cb", name="cb")
            w2p = cb[:, 0:2]
            bm8 = cb[:, 2:10]
            bm16 = cb[:, 10:26].rearrange("p (b h) -> p b h", h=H)
            wcc = [[cb[:, 26 + (h * 2 + dh) * D:26 + (h * 2 + dh + 1) * D]
                    for dh in range(2)] for h in range(2)]
            bm8T = cb[0:8, 1050:1178]
            if apply_b1:
                b1c = cpool.tile([128, 1], F32, tag="b1c")
                nc.vector.tensor_copy(b1c[:], cb[:, 1818:1819])
            gam_t = (
                cload("gam", gam_d[:].to_broadcast((128, D)), [128, D], F32)
                if apply_gamma_beta else None
            )
            bet_t = (
                cload("bet", bet_d[:].to_broadcast((128, D)), [128, D], F32)
                if apply_gamma_beta else None
            )

            def issue_loads(t):
                blob = lpool.tile([128, TBLOB], F8, tag="blob")
                nc.sync.dma_start(blob[:], blob_d[t])
                neT = blob[:, 0:4096].rearrange("p (i c) -> p i c", i=2)
                ner = blob[:, 4096:8192].rearrange("p (c d) -> p c d", c=NCH)
                cen = blob[:, 8192:8704].bitcast(F16)
                nwv = blob[:, 8704:8736].bitcast(F16)
                return neT, ner, cen, nwv

            def issue_fused(t, aggT, cen, ps=None):
                # fused = combined @ Wc (bc folded into cen host-side)
                fu_ps = ps if ps is not None else fups_p.tile(
                    [128, D], F32, tag="fu")
                mms = [(h, dh) for h in range(2) for dh in range(2)]
                for i, (h, dh) in enumerate(mms):
                    lhs = aggT[:, dh].rearrange("p (b h) -> p h b", h=2)[:, h, :]
                    nc.tensor.matmul(
                        fu_ps[:], lhs, wcc[h][dh],
                        start=(i == 0), stop=(i == 3),
                    )
                return (t, fu_ps, cen)

            # LN stats staged per-tile into columns; the tiny rsqrt chain
            # runs BATCHED (4 tiles per group) on the otherwise-idle GPSIMD
            # engine so it never clogs the DVE queue (the rawn->exp->S
            # critical chain was stalling ~1us/tile behind it).
            GRP = 4
            xts = {}
            gstats = {}

            def issue_ln_pre(t, fu_ps, cen):
                # residual add + sum / sumsq accumulation (DVE)
                g = t // GRP
                if g not in gstats:
                    gstats[g] = (
                        gpool.tile([128, GRP], F32, tag="msum_g",
                                   name="msum_g"),
                        gpool.tile([128, GRP], F32, tag="sumsq_g",
                                   name="sumsq_g"),
                        gpool.tile([128, GRP], F32, tag="negm_g",
                                   name="negm_g"),
                        gpool.tile([128, GRP], I32, tag="inv_g",
                                   name="inv_g"),
                    )
                msum_g, sumsq_g, _, _ = gstats[g]
                i = t % GRP
                x_t = xpool.tile([128, D], F32, tag="x")
                xts[t] = x_t
                nc.vector.scalar_tensor_tensor(
                    x_t[:], fu_ps[:], 1.0, cen[:],
                    op0=ALU.mult, op1=ALU.add, accum_out=msum_g[:, i:i + 1],
                )
                if debug and t == 0:
                    nc.sync.dma_start(dbg_x_d[:], x_t[:])
                sq_t = wpool.tile([128, D], F32, tag="sq")
                nc.vector.scalar_tensor_tensor(
                    sq_t[:], x_t[:], 1.0, x_t[:],
                    op0=ALU.mult, op1=ALU.mult,
                    accum_out=sumsq_g[:, i:i + 1],
                )

            def issue_ln_chain(g, dve=False):
                # invstd = rsqrt(sumsq/D - mean^2 + eps) for 4 tiles at once.
                # Seed comes from sumsq/D + eps alone (the mean^2 term is
                # ~1% of q for LN inputs; Newton recovers it) so the DVE ops
                # here never depend on GPSIMD results. The exact q and the
                # Newton steps run on the otherwise-idle GPSIMD.
                msum_g, sumsq_g, negm_g, inv_g = gstats[g]
                negm = negm_g[:]
                yi = inv_g[:]
                q_t = gpool.tile([128, GRP], F32, tag="qg")
                qh = gpool.tile([128, GRP], F32, tag="qhg")
                m2 = gpool.tile([128, GRP], F32, tag="m2g")
                nr1 = gpool.tile([128, GRP], F32, tag="nr1g")
                nr2 = gpool.tile([128, GRP], F32, tag="nr2g")
                nc.vector.tensor_scalar(
                    q_t[:], sumsq_g[:], 1.0 / D, EPS,
                    op0=ALU.mult, op1=ALU.add,
                )
                nc.vector.tensor_scalar(
                    yi, q_t[:].bitcast(I32), 1, None,
                    op0=ALU.logical_shift_right,
                )
                nc.vector.tensor_scalar(
                    yi, yi, RSQRT_MAGIC, -1, op0=ALU.subtract, op1=ALU.mult,
                )
                E = nc.vector if dve else nc.gpsimd
                E.tensor_scalar_mul(negm, msum_g[:], -1.0 / D)
                E.tensor_mul(m2[:], negm, negm)
                E.tensor_sub(q_t[:], q_t[:], m2[:])
                E.tensor_scalar_mul(qh[:], q_t[:], -0.5)
                y = yi.bitcast(F32)
                for _ in range(2):
                    E.tensor_mul(nr1[:], y, y)
                    E.tensor_mul(nr2[:], qh[:], nr1[:])
                    E.tensor_scalar(nr1[:], nr2[:], 1.5, None, op0=ALU.add)
                    E.tensor_mul(yi.bitcast(F32), y, nr1[:])

            def issue_ln_post(t):
                g = t // GRP
                i = t % GRP
                _, _, negm_g, inv_g = gstats[g]
                x_t = xts.pop(t)
                xn = wpool.tile([128, D], F32, tag="xn")
                nc.vector.tensor_scalar(
                    xn[:], x_t[:], negm_g[:, i:i + 1],
                    inv_g[:, i:i + 1].bitcast(F32),
                    op0=ALU.add, op1=ALU.mult,
                )
                if apply_gamma_beta:
                    nc.vector.tensor_mul(xn[:], xn[:], gam_t[:])
                    nc.vector.tensor_add(xn[:], xn[:], bet_t[:])
                nc.sync.dma_start(out_d[t * 128:(t + 1) * 128, :], xn[:])

            def issue_z_vu(t, neT, hf):
                # z partial: VU.T @ neT8, fp8 DoubleRow, 512-col MMs (one
                # PSUM bank each). All vu MMs for both halves are grouped so
                # the DoubleRow weights load only twice per tile.
                z_ps = zps_p.tile([128, 1024], F32, tag="z")
                for m in range(2):
                    c0 = hf * 1024 + m * 512
                    nc.tensor.matmul(
                        z_ps[:, m * 512:(m + 1) * 512], vu8,
                        neT[:, :, c0:c0 + 512],
                        start=True, stop=False, perf_mode=PM.DoubleRow,
                        skip_group_check=True,
                    )
                return z_ps

            def issue_z_u2(t, z_ps, hf):
                # z accumulate: U2.T @ ceT8-kexp (stride-0 k-broadcast AP)
                for m in range(2):
                    b0 = t * 128 + hf * 64 + m * 32
                    ce_kexp = ceT8[:, :, b0:b0 + 32][:, :, :, None] \
                        .to_broadcast((128, 2, 32, 16))
                    nc.tensor.matmul(
                        z_ps[:, m * 512:(m + 1) * 512], u28, ce_kexp,
                        start=False, stop=True, perf_mode=PM.DoubleRow,
                        skip_group_check=True,
                    )

            def issue_tanh(h_sb, z_ps, hf):
                if apply_b1:
                    nc.scalar.activation(
                        h_sb[:, hf * 1024:(hf + 1) * 1024], z_ps[:],
                        AFT.Tanh, bias=b1c[:],
                    )
                else:
                    nc.scalar.activation(
                        h_sb[:, hf * 1024:(hf + 1) * 1024], z_ps[:], AFT.Tanh,
                    )

            # ---- PE warmup spin ----
            # The HAM clock gate keeps the PE at 1.2 GHz until it sees
            # ~3.4us of sustained matmul activity. Spin dummy 256-col MMs
            # on scratch data during the initial blob DMA so real work
            # starts at 2.4 GHz. Output goes to the fused PSUM bank and is
            # overwritten (start=True) by the first real fused matmul.
            warm_sb = cpool.tile([128, 256], F8, tag="warm", name="warm")
            nc.gpsimd.memset(warm_sb[:], 0)
            warm_ps = fups_p.tile([128, 256], F32, tag="fu")
            for _ in range(24):
                nc.tensor.matmul(
                    warm_ps[:], warm_sb[:, 0:128], warm_sb[:],
                    start=True, stop=True, skip_group_check=True,
                )

            def issue_raw(t, h_sb):
                # raw scores (chunk-stationary), + nwv add, + exp.
                # Issued at the tail of iteration t-1 so em(t) is ready
                # before S(t) at the start of iteration t.
                srm = srm_p.tile([128, NCH, H], F32, tag="srm")
                for c in range(NCH):
                    nc.tensor.matmul(
                        srm[:, c, :],
                        h_sb[:, c * 128:(c + 1) * 128], w2p,
                        start=True, stop=True,
                    )
                nwv = tiles[t][3]
                rawn = wpool.tile([128, NCH, H], F32, tag="rawn")
                nc.vector.tensor_add(
                    rawn[:], srm[:],
                    nwv[:, :, None].to_broadcast((128, NCH, H)),
                )
                if debug and t == 0:
                    nc.sync.dma_start(dbg_rawn_d[:], rawn[:])
                em = wpool.tile([128, NCH, H], F16, tag="em")
                nc.scalar.activation(
                    em[:].rearrange("p c h -> p (c h)"),
                    rawn[:].rearrange("p c h -> p (c h)"), AFT.Exp,
                )
                return srm, em

            # ---- software-pipelined main loop ----
            # iteration t: S/softmax/agg/copy for tile t, fused+LN-pre for
            # t-2, z+tanh for t+1, raw+exp for t+1 at the tail (after agg,
            # whose 32 MMs hide the tanh-h1 latency). em(t) is produced at
            # the previous iteration's tail so S(t) never waits; the aggT
            # copy goes last in the ACT queue (it isn't needed for 2 more
            # iterations) so exp(t+1) retires promptly.
            tiles = {0: issue_loads(0)}
            nc.sync.dma_start(ceT8[:], ceT_d[:])
            nc.sync.dma_start(cb[:], cb_d[:])
            tiles[1] = issue_loads(1)
            hs = {0: wpool.tile([128, 2048], F16, tag="h", name="h")}
            zpa = issue_z_vu(0, tiles[0][0], 0)
            zpb = issue_z_vu(0, tiles[0][0], 1)
            issue_z_u2(0, zpa, 0)
            issue_tanh(hs[0], zpa, 0)
            issue_z_u2(0, zpb, 1)
            issue_tanh(hs[0], zpb, 1)
            if debug:
                nc.sync.dma_start(dbg_h_d[:], hs[0][:])
            srm_cur, em_cur = issue_raw(0, hs[0])
            pending = []
            ready_posts = []
            copy_pending = None
            zprev1 = zprev2 = None
            for t in range(NBT):
                neT, ner, cen, nwv = tiles[t]
                if t + 2 < NBT:
                    tiles[t + 2] = issue_loads(t + 2)
                srm, em = srm_cur, em_cur

                ln_args = None
                if len(pending) == 2:
                    ln_args = issue_fused(*pending.pop(0))
                # aggT copy of tile t-1, deferred to AFTER fused(t-2)'s
                # issue: the scheduler then orders it late in the ACT queue,
                # so fused's (coalesced) ACT-sem wait never covers it. A
                # full iteration remains before fused(t-1) needs it.
                if copy_pending is not None:
                    nc.scalar.copy(copy_pending[0][:], copy_pending[1][:])
                    copy_pending = None

                # keep-warm filler: zero-matmuls (warm_sb is memset 0) into
                # the consumed z banks of tile t (tanh already read them).
                # Accumulate +0 with start=False: value-safe, no cross-engine
                # deps; keeps the PE clock gate at 2.4GHz through the S and
                # recS waits.
                if zprev1 is not None:
                    for j in range(5):
                        nc.tensor.matmul(
                            zprev1[:, j * 64:(j + 1) * 64],
                            warm_sb[:, 0:128], warm_sb[:, 0:64],
                            start=False, stop=False, skip_group_check=True,
                        )

                s_ps = srm[0:8].rearrange("p c h -> p (c h)")
                nc.tensor.matmul(
                    s_ps, bm8, em[:].rearrange("p c h -> p (c h)"),
                    start=True, stop=True,
                )
                s_eps = wpool.tile([8, NCH * H], F32, tag="seps")
                nc.vector.tensor_scalar_add(s_eps[:], s_ps, S_EPS)
                recS = wpool.tile([8, NCH * H], F16, tag="recS")
                with nc.allow_low_precision(reason="recS feeds fp16 matmul"):
                    nc.vector.reciprocal(recS[:], s_eps[:])

                # z-vu for both halves of t+1 fills the recS latency on PE
                # (single DoubleRow weight load for all 4 MMs)
                zp1 = zp2 = None
                if t + 1 < NBT:
                    hs[t + 1] = wpool.tile([128, 2048], F16, tag="h",
                                           name="h")
                    zp1 = issue_z_vu(t + 1, tiles[t + 1][0], 0)
                    zp2 = issue_z_vu(t + 1, tiles[t + 1][0], 1)

                if zprev2 is not None:
                    for j in range(5):
                        nc.tensor.matmul(
                            zprev2[:, j * 64:(j + 1) * 64],
                            warm_sb[:, 0:128], warm_sb[:, 0:64],
                            start=False, stop=False, skip_group_check=True,
                        )

                rmap_ps = srm[:].rearrange("p c h -> p (c h)")
                nc.tensor.matmul(rmap_ps, bm8T, recS[:], start=True, stop=True)

                # z-u2 for both halves fills the p/expblk latency before agg
                # (single u2 weight load); tanh after each half completes
                if t + 1 < NBT:
                    issue_z_u2(t + 1, zp1, 0)
                    issue_tanh(hs[t + 1], zp1, 0)
                    issue_z_u2(t + 1, zp2, 1)
                    issue_tanh(hs[t + 1], zp2, 1)

                p_sb = wpool.tile([128, NCH, H], F16, tag="p")
                nc.vector.tensor_mul(p_sb[:], em[:], srm[:])
                if debug and t == 0:
                    nc.sync.dma_start(dbg_em_d[:], em[:])
                    nc.sync.dma_start(dbg_p_d[:], p_sb[:])
                expblk = wpool.tile([128, NCH, 8, H], F16, tag="expblk")
                for half in range(2):
                    hc = slice(half * (NCH // 2), (half + 1) * (NCH // 2))
                    nc.vector.tensor_mul(
                        expblk[:, hc],
                        p_sb[:, hc, None, :].to_broadcast((128, NCH // 2, 8, H)),
                        bm16[:, None, :, :].to_broadcast((128, NCH // 2, 8, H)),
                    )
                chain_g = None
                if ln_args is not None:
                    issue_ln_pre(*ln_args)
                    tp = ln_args[0]
                    if tp % GRP == GRP - 1:
                        chain_g = tp // GRP
                # last iteration: pull fused(NBT-2) forward (its ln_pre for
                # NBT-4 is already issued above, so the fu_ps WAR is safe)
                ln_extra = None
                if t == NBT - 1 and pending:
                    ln_extra = issue_fused(*pending.pop(0))
                # ---- aggT[dd, dh, (b,h)] += ner8_c.T @ expblk_c ----
                agg_ps = aggps_p.tile([128, 2, 2 * 128], F32, tag="aggT")
                for c in range(NCH):
                    for dh in range(2):
                        nc.tensor.matmul(
                            agg_ps[:, dh, 16 * c:16 * c + 16],
                            ner[:, c, dh * 128:(dh + 1) * 128],
                            expblk[:, c],
                            start=True, stop=True,
                        )
                # raw/exp for t+1 at the tail: agg's MMs just filled the
                # tanh(t+1,h1) latency, so raw doesn't stall
                if t + 1 < NBT:
                    srm_cur, em_cur = issue_raw(t + 1, hs[t + 1])
                # rsqrt chain + xn posts AFTER rawn(t+1) in the DVE queue:
                # on group-boundary iterations their seed ops were delaying
                # rawn -> exp -> S(t+1) by ~0.8us, which tripped the PE
                # clock gate for the rest of the kernel
                if chain_g is not None:
                    issue_ln_chain(chain_g)
                    # delay posts 2 iterations so the GPSIMD chain is
                    # long done before any DVE op waits on it
                    for j, tpost in enumerate(
                            range(GRP * chain_g, GRP * chain_g + GRP)):
                        ready_posts.append((t + 2 + j, tpost))
                while ready_posts and ready_posts[0][0] <= t:
                    issue_ln_post(ready_posts.pop(0)[1])
                aggT = wpool.tile([128, 2, 2 * 128], F16, tag="aggTsb", bufs=3)
                copy_pending = (aggT, agg_ps)
                if debug and t == 0:
                    nc.sync.dma_start(dbg_aggT_d[:], aggT[:])
                tiles.pop(t - 2, None)
                hs.pop(t - 1, None)
                pending.append((t, aggT, cen))
                zprev1, zprev2 = zp1, zp2

            # drain: ln_pre for NBT-2 (its fused was pulled into the last
            # iteration), then fused(NBT-1) into a now-free z PSUM bank (so
            # it doesn't wait on the fu_ps WAR), the last rsqrt chain on DVE
            # (GPSIMD dispatch is slow and nothing else needs DVE now), and
            # the remaining posts oldest-first.
            if copy_pending is not None:
                nc.scalar.copy(copy_pending[0][:], copy_pending[1][:])
                copy_pending = None
            if ln_extra is not None:
                issue_ln_pre(*ln_extra)
            for args in pending:
                fu15 = zps_p.tile([128, 1024], F32, tag="z", name="fu15")
                issue_ln_pre(*issue_fused(*args, ps=fu15[:, 0:D]))
                tp = args[0]
                if tp % GRP == GRP - 1:
                    issue_ln_chain(tp // GRP, dve=True)
                    for tpost in range(tp - GRP + 1, tp + 1):
                        ready_posts.append((0, tpost))
            for _, tp in ready_posts:
                issue_ln_post(tp)

    nc.finalize()
    return nc


def _f8(x):
    import ml_dtypes
    return np.clip(x, -240.0, 240.0).astype(ml_dtypes.float8_e4m3)


def _patch_ldw_opt():
    import concourse.bass_utils as _bu
    if getattr(_bu, "_ldwopt_patched", False):
        return
    _bu._ldwopt_patched = True


def kernel(center_emb, neighbor_embs, neighbor_weights, neighbor_valid,
           W1, b1, w2, Wc, bc, alpha, gamma, beta):
    _patch_ldw_opt()
    from concourse.bass_utils import run_bass_kernel_spmd

    global LAST_EXEC_NS

    f32 = np.float32
    f16 = np.float16
    ce = np.asarray(center_emb, f32)
    ne = np.asarray(neighbor_embs, f32)
    nw = np.asarray(neighbor_weights, f32)
    va = np.asarray(neighbor_valid)
    W1 = np.asarray(W1, f32)
    b1 = np.asarray(b1, f32)
    w2 = np.asarray(w2, f32)
    Wc = np.asarray(Wc, f32)
    bc = np.asarray(bc, f32)
    alpha = np.asarray(alpha, f32)
    gamma = np.asarray(gamma, f32)
    beta = np.asarray(beta, f32)

    apply_gamma_beta = not (np.all(gamma == 1.0) and np.all(beta == 0.0))
    apply_b1 = bool(np.any(b1 != 0.0))

    key = (apply_gamma_beta, apply_b1, bool(os.environ.get("NE_DEBUG_DUMP")))
    if key not in _prog_cache:
        _prog_cache[key] = _build_program(key[0], key[1])
    nc = _prog_cache[key]

    # ---- host-side const prep (weight folding + dtype casts + layouts) ----
    import ml_dtypes
    F8NP = ml_dtypes.float8_e4m3
    sig = 1.0 / (1.0 + np.exp(-float(alpha[0])))
    VU = np.concatenate([W1[h, D:2 * D] - W1[h, 2 * D:3 * D] for h in range(H)], axis=1)
    U2 = np.concatenate([W1[h, :D] + W1[h, 2 * D:3 * D] for h in range(H)], axis=1)
    # d = p + 128*i  ->  [p, i, cols];  vu | u2 side by side
    vu2 = np.concatenate([
        _f8(VU).reshape(2, 128, HA).transpose(1, 0, 2),
        _f8(U2).reshape(2, 128, HA).transpose(1, 0, 2),
    ], axis=2)
    vu2 = np.ascontiguousarray(vu2)

    cb = np.zeros((128, CBLOB), f16)
    for h in range(H):
        cb[h * A:(h + 1) * A, h] = w2[h].astype(f16)          # w2p [*,0:2]
    pidx = np.arange(128)
    cb[:, 2:10] = (pidx[:, None] // 16 == np.arange(8)[None, :])   # bm8
    for p in range(128):
        cb[p, 10 + (p // 16) * H:10 + (p // 16) * H + H] = 1.0     # bm16
    wcc_f = (Wc * sig).astype(f16).reshape(H, 2, 128, D)
    for h in range(H):
        for dh in range(2):
            c0 = 26 + (h * 2 + dh) * D
            cb[:, c0:c0 + D] = wcc_f[h, dh]
    cb[0:8, 1050:1178] = (pidx[None, :] // 16 == np.arange(8)[:, None])  # bm8T
    for pl in range(32):
        cb[pl, 1178 + pl * 16:1178 + (pl + 1) * 16] = 1.0       # idk
    cb[0, 1690:1818] = 1.0                                       # ones_row
    cb[:, 1818] = b1.reshape(HA).astype(f16)                     # b1 column

    gamma_r = gamma.reshape(1, D).astype(f32)
    beta_r = beta.reshape(1, D).astype(f32)

    nwv = np.where(va, nw, NWV_NEG).astype(f16)        # [B, K]
    ce_bc = (ce + (bc * sig)[None, :]).astype(f16)     # bc folded into center

    in_maps = []
    for cidx in range(NCORES):
        rs = slice(cidx * BC, (cidx + 1) * BC)
        ne_c = _f8(ne[rs].reshape(BC * K, D))          # [BC*K, D] fp8
        blob = np.zeros((NBT, 128, TBLOB), np.uint8)
        # neT8 [t, p, i, col]: ne[row(t,col), p+128i]
        neT8 = np.ascontiguousarray(
            ne_c.reshape(NBT, 2048, 2, 128).transpose(0, 3, 2, 1)
        )
        blob[:, :, 0:4096] = neT8.reshape(NBT, 128, 4096).view(np.uint8)
        # ner8 [t, p, c, d]: ne[t*2048 + c*128 + p, d]
        ner8 = np.ascontiguousarray(
            ne_c.reshape(NBT, NCH, 128, D).transpose(0, 2, 1, 3)
        )
        blob[:, :, 4096:8192] = ner8.reshape(NBT, 128, 4096).view(np.uint8)
        blob[:, :, 8192:8704] = np.ascontiguousarray(
            ce_bc[rs].reshape(NBT, 128, D)).view(np.uint8).reshape(NBT, 128, 512)
        blob[:, :, 8704:8736] = np.ascontiguousarray(
            nwv[rs].reshape(NBT, NCH, 128).transpose(0, 2, 1)
        ).view(np.uint8).reshape(NBT, 128, 32)
        ceT8 = np.ascontiguousarray(
            _f8(ce[rs]).reshape(BC, 2, 128).transpose(2, 1, 0)
        )
        in_maps.append({
            "blob8": blob.view(F8NP),
            "ceT8": ceT8,
            "vu2_8": vu2,
            "cblob16": cb,
            "gamma_r": gamma_r,
            "beta_r": beta_r,
        })

    trace = bool(os.environ.get("NE_KERNEL_TRACE"))
    if trace:
        _maybe_install_profile_hook()
    res = run_bass_kernel_spmd(nc, in_maps, list(range(NCORES)), trace=trace)
    LAST_EXEC_NS = res.exec_time_ns
    if trace:
        print("kernel exec_time_ns:", res.exec_time_ns, "mean:", res.mean_exec_time_ns)

    out = np.empty((B, D), f32)
    for cidx in range(NCORES):
        out[cidx * BC:(cidx + 1) * BC] = res.results[cidx]["out"]
    return out

